# revision 4
# baseline (speedup 1.0000x reference)
"""BertSelfAttention (relative_key_query) Trainium2 Bass kernel.

Sharding: 8 cores = 4 batches x 2 head-groups (6 heads each).
Each core computes, for its (batch b, heads hg*6..hg*6+6):
  q/k/v projections, scores^T = k q^T + Toeplitz positional biases,
  softmax (via exp + ones-column sums in the ctx matmul), ctx = probs @ v.

Orientation: scores are built TRANSPOSED (r on partitions, l on free dim) so
the probs tensor directly feeds the ctx matmul as lhsT (contraction over r).

Toeplitz bias terms bias_q[l,r] = q[l].D[l-r+1023], bias_k[l,r] = k[r].D[...]:
  - per r-tile/l-tile "window" matmuls  QD/KD = q/k @ D_slice^T  -> DRAM
  - diagonal (stride pitch-1) DMA reads realign windows into (r,l) tiles:
      bias_k: plain strided dma_start  (per-partition offset via DRAM strides)
      bias_q: dma_gather(transpose=True) with elem_step = pitch-1
  - bias_k is accumulated onto the gathered bias_q via SWDGE accum_op=add
  - combined bias is injected into the scores PSUM via an identity matmul
    (start=True) before the qk matmuls accumulate (start=False).
"""

import sys

sys.path.insert(0, "/opt/trn_rl_repo")

import numpy as np

import concourse.bass as bass
import concourse.mybir as mybir
from concourse import bacc, library_config
from concourse.tile import TileContext

S = 1024
HID = 768
DH = 64
HPC = 6  # heads per core
NP = 3  # head pairs per core
NT = 8  # 128-row tiles per S
PITCH = 1153  # DRAM window row pitch (elements)
STEP = 1152  # diagonal stride = PITCH-1; f16 bytes 2304 = 9*256 (dma_gather ok)
WW = 1151  # window valid width
DPADC = 2176  # padded D-operand columns (896 + 1280)
WCH = [(0, 512), (512, 512), (1024, 256)]  # window matmul N-chunks

F32 = mybir.dt.float32
F32R = mybir.dt.float32r
F16 = mybir.dt.float16
I16 = mybir.dt.int16
AF = mybir.ActivationFunctionType
ALU = mybir.AluOpType


def _emit(nc, tc, T, rep):
    r = f"_{rep}"
    with tc.tile_pool(name="const" + r, bufs=1) as cp:
        d2r_sb = cp.tile([128, DPADC], F32R)
        nc.sync.dma_start(out=d2r_sb[:], in_=T["d2rev"][:])
        d2f_sb = cp.tile([128, DPADC], F32R)
        nc.sync.dma_start(out=d2f_sb[:], in_=T["d2fwd"][:])
        id_sb = cp.tile([128, 128], F16)
        nc.sync.dma_start(out=id_sb[:], in_=T["ident"][:])
        ix_sb = cp.tile([128, 8], I16)
        nc.sync.dma_start(out=ix_sb[:], in_=T["idxs"][:])
        qT_sb = cp.tile([128, NP, S], F32R)
        kT_sb = cp.tile([128, NP, S], F32R)
        v_sb = cp.tile([128, NT, HPC, 80], F16)
        ctxlr = cp.tile([128, NT, HPC, 80], F16)
        out_sb = cp.tile([128, NT, 384], F32)

        nc.vector.memset(v_sb[:], 0.0)
        nc.vector.memset(v_sb[:, :, :, 64:65], 1.0)

        # ---- Phase A: projections ----
        with (
            tc.tile_pool(name="ld" + r, bufs=1) as ld,
            tc.tile_pool(name="pps" + r, bufs=2, space="PSUM") as pps,
        ):
            hT_sb = ld.tile([128, 6, S], F32R)
            nc.sync.dma_start(out=hT_sb[:], in_=T["hT"][:])
            wq_sb = ld.tile([128, 6, 384], F32R)
            nc.sync.dma_start(out=wq_sb[:], in_=T["wqT"][:])
            wk_sb = ld.tile([128, 6, 384], F32R)
            nc.sync.dma_start(out=wk_sb[:], in_=T["wkT"][:])
            wv_sb = ld.tile([128, 6, 384], F32R)
            nc.sync.dma_start(out=wv_sb[:], in_=T["wvT"][:])

            for wsb, dst in ((wq_sb, qT_sb), (wk_sb, kT_sb)):
                for m in range(3):
                    for n in range(2):
                        ps = pps.tile([128, 512], F32, tag="pp")
                        for kk in range(6):
                            nc.tensor.matmul(
                                ps[:],
                                lhsT=wsb[:, kk, 128 * m : 128 * (m + 1)],
                                rhs=hT_sb[:, kk, 512 * n : 512 * (n + 1)],
                                start=(kk == 0),
                                stop=(kk == 5),
                            )
                        if (m + n) % 2:
                            nc.scalar.copy(dst[:, m, 512 * n : 512 * (n + 1)], ps[:])
                        else:
                            nc.vector.tensor_copy(
                                dst[:, m, 512 * n : 512 * (n + 1)], ps[:]
                            )
            for s in range(NT):
                ps = pps.tile([128, 384], F32, tag="pv")
                for kk in range(6):
                    nc.tensor.matmul(
                        ps[:],
                        lhsT=hT_sb[:, kk, 128 * s : 128 * (s + 1)],
                        rhs=wv_sb[:, kk, :],
                        start=(kk == 0),
                        stop=(kk == 5),
                    )
                nc.scalar.copy(
                    v_sb[:, s, :, 0:64],
                    ps[:].rearrange("p (h d) -> p h d", h=HPC),
                )

        # ---- Phase B: QD/KD windows to DRAM ----
        qwin = {}
        kwin = {}
        with (
            tc.tile_pool(name="wps" + r, bufs=2, space="PSUM") as wps,
            tc.tile_pool(name="wsb" + r, bufs=4) as wsb,
            tc.tile_pool(name="wdram" + r, bufs=HPC, space="DRAM") as wdram,
        ):
            for p in range(NP):
                for t in range(NT):
                    base = 896 - 128 * t
                    for srcT, d2, store, nm in (
                        (qT_sb, d2r_sb, qwin, "q"),
                        (kT_sb, d2f_sb, kwin, "k"),
                    ):
                        psA = wps.tile([128, 1280], F32, tag="w")
                        psB = wps.tile([128, 1280], F32, tag="w")
                        for c0, cn in WCH:
                            nc.tensor.matmul(
                                psA[:, c0 : c0 + cn],
                                lhsT=srcT[0:64, p, 128 * t : 128 * (t + 1)],
                                rhs=d2[0:64, base + c0 : base + c0 + cn],
                                start=True,
                                stop=True,
                                tile_position=(0, 0),
                            )
                            nc.tensor.matmul(
                                psB[:, c0 : c0 + cn],
                                lhsT=srcT[64:128, p, 128 * t : 128 * (t + 1)],
                                rhs=d2[64:128, base + c0 : base + c0 + cn],
                                start=True,
                                stop=True,
                                tile_position=(64, 0),
                            )
                        for ps, h in ((psA, 2 * p), (psB, 2 * p + 1)):
                            wt = wsb.tile([128, PITCH], F16, tag="wsb")
                            if h % 2:
                                nc.scalar.copy(wt[:], ps[:, 0:PITCH])
                            else:
                                nc.vector.tensor_copy(wt[:], ps[:, 0:PITCH])
                            dtile = wdram.tile([128, PITCH], F16, tag=f"{nm}{t}")
                            nc.sync.dma_start(out=dtile[:], in_=wt[:])
                            store[(h, t)] = dtile

        # ---- Phase C: gathers + scores + softmax + ctx ----
        with (
            tc.tile_pool(name="g" + r, bufs=2) as gpool,
            tc.tile_pool(name="sps" + r, bufs=2, space="PSUM") as sps,
            tc.tile_pool(name="cps" + r, bufs=2, space="PSUM") as cps,
            tc.tile_pool(name="pt" + r, bufs=6) as ptp,
            tc.tile_pool(name="sm" + r, bufs=4) as smp,
        ):
            for p in range(NP):
                bqg = {}
                for h in (2 * p, 2 * p + 1):
                    g = gpool.tile([128, NT, NT, 128], F16, tag="bqg")
                    bqg[h] = g
                    for t in range(NT):
                        w = qwin[(h, t)]
                        src = bass.AP(
                            w.tensor, w.offset + 127, [[STEP, 128], [1, 1024]]
                        )
                        nc.gpsimd.dma_gather(
                            g[:, t, :, :], src, ix_sb[:], 128, 128, 1024,
                            elem_step=STEP, transpose=True,
                        )
                    for c in range(NT):
                        w = kwin[(h, c)]
                        src = bass.AP(
                            w.tensor, w.offset + 127, [[STEP, 128], [1, 1024]]
                        )
                        nc.gpsimd.dma_start(
                            out=g[:, :, c, :], in_=src, accum_op=ALU.add
                        )
                ctxps = {}
                for h in (2 * p, 2 * p + 1):
                    ctx_t = cps.tile([80, 1024], F32, tag="ctx")
                    ctxps[h] = ctx_t
                for s in range(NT):
                    sc = {}
                    for h in (2 * p, 2 * p + 1):
                        sc_t = sps.tile([128, 1024], F32, tag="sc")
                        sc[h] = sc_t
                    for n in (0, 1):
                        lo, hi = 512 * n, 512 * (n + 1)
                        for h in (2 * p, 2 * p + 1):
                            nc.tensor.matmul(
                                sc[h][:, lo:hi],
                                lhsT=id_sb[:],
                                rhs=bqg[h][:, 4 * n : 4 * n + 4, s, :],
                                start=True,
                                stop=False,
                            )
                        nc.tensor.matmul(
                            sc[2 * p][:, lo:hi],
                            lhsT=kT_sb[0:64, p, 128 * s : 128 * (s + 1)],
                            rhs=qT_sb[0:64, p, lo:hi],
                            start=False,
                            stop=True,
                            tile_position=(0, 0),
                        )
                        nc.tensor.matmul(
                            sc[2 * p + 1][:, lo:hi],
                            lhsT=kT_sb[64:128, p, 128 * s : 128 * (s + 1)],
                            rhs=qT_sb[64:128, p, lo:hi],
                            start=False,
                            stop=True,
                            tile_position=(64, 0),
                        )
                    for h in (2 * p, 2 * p + 1):
                        pt = ptp.tile([128, 1024], F16, tag="pt")
                        nc.scalar.activation(pt[:], sc[h][:], AF.Exp, scale=0.125)
                        for n in (0, 1):
                            lo, hi = 512 * n, 512 * (n + 1)
                            nc.tensor.matmul(
                                ctxps[h][:, lo:hi],
                                lhsT=v_sb[:, s, h, 0:80],
                                rhs=pt[:, lo:hi],
                                start=(s == 0),
                                stop=(s == NT - 1),
                            )
                # tail per head: evac ctx^T, transpose to (l, dh), divide
                for h in (2 * p, 2 * p + 1):
                    cte = smp.tile([80, 1024], F16, tag="cte")
                    nc.scalar.copy(cte[:], ctxps[h][:])
                    for t in range(NT):
                        nc.sync.dma_start_transpose(
                            ctxlr[:, t, h, :], cte[:, 128 * t : 128 * (t + 1)]
                        )
                    for t in range(NT):
                        rc = smp.tile([128, 1], F32, tag="rc")
                        nc.vector.reciprocal(rc[:], ctxlr[:, t, h, 64:65])
                        nc.vector.tensor_scalar(
                            out_sb[:, t, 64 * h : 64 * (h + 1)],
                            ctxlr[:, t, h, 0:64],
                            rc[:],
                            None,
                            ALU.mult,
                        )
        nc.sync.dma_start(
            out=T["out"][:].rearrange("(t p) c -> p t c", p=128), in_=out_sb[:]
        )


def build_nc(reps=1):
    nc = bacc.Bacc("TRN2", target_bir_lowering=False)
    T = {
        "hT": nc.dram_tensor("hT", [6, 128, S], F32R, kind="ExternalInput"),
        "wqT": nc.dram_tensor("wqT", [6, 128, 384], F32R, kind="ExternalInput"),
        "wkT": nc.dram_tensor("wkT", [6, 128, 384], F32R, kind="ExternalInput"),
        "wvT": nc.dram_tensor("wvT", [6, 128, 384], F32R, kind="ExternalInput"),
        "d2rev": nc.dram_tensor("d2rev", [128, DPADC], F32R, kind="ExternalInput"),
        "d2fwd": nc.dram_tensor("d2fwd", [128, DPADC], F32R, kind="ExternalInput"),
        "ident": nc.dram_tensor("ident", [128, 128], F16, kind="ExternalInput"),
        "idxs": nc.dram_tensor("idxs", [128, 8], I16, kind="ExternalInput"),
        "out": nc.dram_tensor("out", [S, 384], F32, kind="ExternalOutput"),
    }
    with TileContext(nc) as tc:
        nc.gpsimd.load_library(library_config.mlp)
        for rep in range(reps):
            _emit(nc, tc, T, rep)
    nc.compile()
    return nc


def host_inputs(hidden_states, Wq, Wk, Wv, dist_emb):
    """Per-core input dicts (layout-only numpy prep)."""
    hidden_states = np.asarray(hidden_states, dtype=np.float32)
    Wq = np.asarray(Wq, dtype=np.float32)
    Wk = np.asarray(Wk, dtype=np.float32)
    Wv = np.asarray(Wv, dtype=np.float32)
    D = np.asarray(dist_emb, dtype=np.float32)

    dfwd = np.zeros((64, DPADC), np.float32)
    dfwd[:, : 2 * S - 1] = D.T
    drev = np.zeros((64, DPADC), np.float32)
    drev[:, : 2 * S - 1] = D[::-1].T
    d2fwd = np.ascontiguousarray(np.concatenate([dfwd, dfwd], axis=0))
    d2rev = np.ascontiguousarray(np.concatenate([drev, drev], axis=0))
    ident = np.eye(128, dtype=np.float16)
    idxs = np.zeros((128, 8), np.int16)
    for pp in range(128):
        for ss in range(8):
            idxs[pp, ss] = 16 * ss + (pp % 16)

    in_maps = []
    for c in range(8):
        b, hg = c // 2, c % 2
        rows = slice(64 * HPC * hg, 64 * HPC * (hg + 1))
        hTc = np.ascontiguousarray(hidden_states[b].T).reshape(6, 128, S)
        in_maps.append(
            {
                "hT": hTc,
                "wqT": np.ascontiguousarray(Wq[rows, :].T).reshape(6, 128, 384),
                "wkT": np.ascontiguousarray(Wk[rows, :].T).reshape(6, 128, 384),
                "wvT": np.ascontiguousarray(Wv[rows, :].T).reshape(6, 128, 384),
                "d2rev": d2rev,
                "d2fwd": d2fwd,
                "ident": ident,
                "idxs": idxs,
            }
        )
    return in_maps


_CACHED_NC = None


def _get_nc():
    global _CACHED_NC
    if _CACHED_NC is None:
        _CACHED_NC = build_nc()
    return _CACHED_NC


def run_cores(in_maps, **kwargs):
    from concourse.bass_utils import run_bass_kernel_spmd

    nc = _get_nc()
    return run_bass_kernel_spmd(nc, in_maps, core_ids=list(range(8)), **kwargs)


def assemble(results):
    out = np.zeros((4, S, HID), np.float32)
    for c in range(8):
        b, hg = c // 2, c % 2
        out[b, :, 384 * hg : 384 * (hg + 1)] = results[c]["out"]
    return out


def kernel(
    hidden_states,
    attention_mask,
    Wq,
    bq,
    Wk,
    bk,
    Wv,
    bv,
    dist_emb,
):
    assert not np.any(np.asarray(attention_mask)), "kernel assumes zero mask"
    assert not (
        np.any(np.asarray(bq)) or np.any(np.asarray(bk)) or np.any(np.asarray(bv))
    ), "kernel assumes zero qkv biases"
    in_maps = host_inputs(hidden_states, Wq, Wk, Wv, dist_emb)
    res = run_cores(in_maps)
    return assemble(res.results)
